# revision 12
# baseline (speedup 1.0000x reference)
"""Trainium2 Bass kernel for the CIN-style bilinear layer:

    out[b, o] = sum_{f,p} x0[b,f] * x[b,p] * W[o,f,p] + bias[o]

i.e. a batch of bilinear forms (outer-product + full-window Conv1d), shapes
B=4096, FIELD=128, H_PREV=128, H_NEXT=256, returned as [B, 256, 1] fp32.

Strategy (data-parallel over 8 NeuronCores, batch sharded 512/core):

The contraction index k=(f,p) has size 16384.  Tile it as 128 K-tiles of 128,
where each K-tile kt=(fb,pb) covers a 16x8 block of (f,p): f = fb*16+fi,
p = pb*8+pi, and the within-tile index is k = fi*8+pi.  The outer-product
operand for one K-tile is then

    A[k, kt, b] = x0T[f(k), b] * xT[p(k), b]

which is an elementwise product of two *small-factor replicated* tensors:
rep8_x0t[k, fb, b] = x0T[fb*16 + k//8, b]   (8x partition replication, 1MB)
rep16_xt[k, pb, b] = xT[pb*8 + k%8, b]      (16x partition replication, 2MB)

Both replications are done on the host (pure data marshalling), so the device
builds A with plain vector tensor_mul ops (fp16, 2x DVE mode, no partition
broadcast needed, no transposes), and the tensor engine runs 128 accumulating
matmuls per batch tile: psum[b,o] += A_kt.T @ W_kt with W pre-permuted on the
host to [fb, k, pb, o].  The bias is added with one K=1 matmul (ones x bias).
Everything on-device is fp16 inputs with fp32 PSUM accumulation.
"""

import numpy as np

import concourse.bacc as bacc
import concourse.mybir as mybir
import concourse.tile as tile
from concourse.bass_utils import run_bass_kernel_spmd

B, F, P, O = 4096, 128, 128, 256
NCORES = 8
BC = B // NCORES  # 512 batch elements per core
FI, PI = 16, 8  # within-K-tile block: k = fi*PI + pi
FB, PB = F // FI, P // PI  # 8 f-blocks, 16 p-blocks; kt = fb*PB + pb
BT = BC // 128  # 4 batch tiles of 128 per core

_F16 = mybir.dt.float16
_F32 = mybir.dt.float32

_NC_CACHE = {}


def _build_nc(repeat=1, loop_n=0):
    """Build + compile the (SPMD, per-core) bass program once.

    repeat>1 re-emits the whole kernel body back-to-back (sharing tile pools,
    so SBUF stays bounded); loop_n>0 additionally wraps the body in a
    hardware For_i loop.  Both are used only by the benchmark harness to
    measure steady-state per-iteration device time via slopes (the axon
    dispatch overhead per call is ~80ms, so single-shot wall time is
    useless).
    """
    key = (repeat, loop_n)
    if key in _NC_CACHE:
        return _NC_CACHE[key]

    nc = bacc.Bacc(
        "TRN2", target_bir_lowering=False, debug=False, num_devices=NCORES
    )

    rep8 = nc.declare_dram_parameter("rep8_x0t", [128, FB, BC], _F16, isOutput=False)
    rep16 = nc.declare_dram_parameter("rep16_xt", [128, PB, BC], _F16, isOutput=False)
    w_re = nc.declare_dram_parameter("w_re", [FB, 128, PB, O], _F16, isOutput=False)
    bias = nc.declare_dram_parameter("bias_col", [O, 1], _F32, isOutput=False)
    # output is stored transposed: out_t[o, b] (host transposes back)
    out = nc.declare_dram_parameter("out_t", [O, BC], _F32, isOutput=True)

    with tile.TileContext(nc) as tc:
        import contextlib

        loop_ctx = (
            tc.For_i(
                0,
                loop_n,
                1,
                hint_engines=(
                    mybir.EngineType.PE,
                    mybir.EngineType.DVE,
                    mybir.EngineType.SP,
                    mybir.EngineType.Activation,
                ),
            )
            if loop_n
            else contextlib.nullcontext()
        )
        with (
            loop_ctx,
            tc.tile_pool(name="inp", bufs=2) as inp,
            tc.tile_pool(name="wp", bufs=2) as wp,
            tc.tile_pool(name="ap", bufs=2) as ap_pool,
            tc.tile_pool(name="op", bufs=2) as op,
            tc.tile_pool(name="ps", bufs=1, space="PSUM") as psp,
        ):
            for _rep in range(repeat):
                rep8_sb = inp.tile([128, FB, BC], _F16, tag="rep8")
                nc.sync.dma_start(rep8_sb[:], rep8[:])
                # split rep16 load in halves so the first tensor_mul can
                # start before the whole 2MB landed
                rep16_sb = inp.tile([128, PB, BC], _F16, tag="rep16")
                nc.sync.dma_start(
                    rep16_sb[:, 0 : PB // 2, :], rep16[:, 0 : PB // 2, :]
                )
                nc.sync.dma_start(
                    rep16_sb[:, PB // 2 : PB, :], rep16[:, PB // 2 : PB, :]
                )
                bias_sb = inp.tile([128, O // 128], _F32, tag="bias")
                for h in range(O // 128):
                    nc.sync.dma_start(
                        bias_sb[:, h : h + 1], bias[h * 128 : (h + 1) * 128, :]
                    )

                # out.T[o, b] accumulators: one full PSUM bank per o-half
                psum_tiles = [
                    psp.tile([128, BC], _F32, tag=f"acc{h}", name=f"acc{h}")
                    for h in range(O // 128)
                ]

                for fb in range(FB):
                    w_sb = wp.tile([128, PB, O], _F16, tag="w", name="w_sb")
                    nc.sync.dma_start(w_sb[:], w_re[fb])

                    a_sb = ap_pool.tile([128, PB, BC], _F16, tag="a", name="a_sb")
                    rep8_bc = rep8_sb[:, fb : fb + 1, :].broadcast_to(
                        (128, PB // 2, BC)
                    )
                    for hh in range(2):
                        sl = slice(hh * (PB // 2), (hh + 1) * (PB // 2))
                        nc.vector.tensor_mul(
                            a_sb[:, sl, :], rep8_bc, rep16_sb[:, sl, :]
                        )

                    for pb in range(PB):
                        for h in range(O // 128):
                            nc.tensor.matmul(
                                psum_tiles[h][:],
                                w_sb[:, pb, h * 128 : (h + 1) * 128],
                                a_sb[:, pb, :],
                                start=(fb == 0 and pb == 0),
                                stop=(fb == FB - 1 and pb == PB - 1),
                            )

                for h in range(O // 128):
                    out_sb = op.tile([128, BC], _F32, tag="out", name="out_sb")
                    # eviction fused with the (exact, fp32) bias add:
                    # out_t[o, b] = psum[o, b] + bias[o]
                    nc.vector.tensor_scalar_add(
                        out_sb[:], psum_tiles[h][:], bias_sb[:, h : h + 1]
                    )
                    nc.sync.dma_start(out[h * 128 : (h + 1) * 128, :], out_sb[:])

    nc.compile()
    _NC_CACHE[key] = nc
    return nc


def _prepare_inputs(x0, x, W, b):
    """Host-side marshalling: cast to fp16, transpose, small-factor replicate,
    permute W, and shard the batch across the 8 cores."""
    x0 = np.asarray(x0, dtype=np.float32)
    x = np.asarray(x, dtype=np.float32)
    W = np.asarray(W, dtype=np.float32)
    b = np.asarray(b, dtype=np.float32)

    x0t = np.ascontiguousarray(x0.T.astype(np.float16))  # [F, B]
    xt = np.ascontiguousarray(x.T.astype(np.float16))  # [P, B]

    # rep8[k, fb, bb] = x0t[fb*FI + k//PI, bb]
    rep8 = np.broadcast_to(
        x0t.reshape(FB, FI, B).transpose(1, 0, 2)[:, None, :, :], (FI, PI, FB, B)
    ).reshape(128, FB, B)
    # rep16[k, pb, bb] = xt[pb*PI + k%PI, bb]
    rep16 = np.broadcast_to(
        xt.reshape(PB, PI, B).transpose(1, 0, 2)[None, :, :, :], (FI, PI, PB, B)
    ).reshape(128, PB, B)

    # w_re[fb, k, pb, o] = W[o, fb*FI + k//PI, pb*PI + k%PI]
    w_re = np.ascontiguousarray(
        W.reshape(O, FB, FI, PB, PI).transpose(1, 2, 4, 3, 0).reshape(FB, 128, PB, O)
    ).astype(np.float16)
    bias_col = b.astype(np.float32).reshape(O, 1)

    in_maps = []
    for c in range(NCORES):
        bs = slice(c * BC, (c + 1) * BC)
        in_maps.append(
            {
                "rep8_x0t": np.ascontiguousarray(rep8[:, :, bs]),
                "rep16_xt": np.ascontiguousarray(rep16[:, :, bs]),
                "w_re": w_re,
                "bias_col": bias_col,
            }
        )
    return in_maps


def kernel(x0, x, W, b, _run_kwargs=None):
    nc = _build_nc()
    in_maps = _prepare_inputs(x0, x, W, b)
    res = run_bass_kernel_spmd(
        nc, in_maps, core_ids=list(range(NCORES)), **(_run_kwargs or {})
    )
    # per-core results are out.T shards [O, BC]; assemble + transpose back
    out_t = np.concatenate(
        [res.results[c]["out_t"] for c in range(NCORES)], axis=1
    )  # [O, B]
    if _run_kwargs:
        kernel._last_results = res
    return np.ascontiguousarray(out_t.T).reshape(B, O, 1).astype(np.float32)


# revision 28
# speedup vs baseline: 1.3625x; 1.3625x over previous
"""Trainium2 Bass kernel for the CIN-style bilinear layer:

    out[b, o] = sum_{f,p} x0[b,f] * x[b,p] * W[o,f,p] + bias[o]

i.e. a batch of bilinear forms (outer-product + full-window Conv1d), shapes
B=4096, FIELD=128, H_PREV=128, H_NEXT=256, returned as [B, 256, 1] fp32.

Strategy (data-parallel over 8 NeuronCores, batch sharded 512/core):

The contraction index k=(f,p) has size 16384.  Tile it as 128 K-tiles of 128,
where each K-tile kt=(fb,pb) covers a 16x8 block of (f,p): f = fb*16+fi,
p = pb*8+pi, and the within-tile index is k = fi*8+pi.  The outer-product
operand for one K-tile is then

    A[k, kt, b] = x0T[f(k), b] * xT[p(k), b]

which is an elementwise product of two *small-factor replicated* tensors:
rep8_x0t[k, fb, b] = x0T[fb*16 + k//8, b]   (8x partition replication, 1MB)
rep16_xt[k, pb, b] = xT[pb*8 + k%8, b]      (16x partition replication, 2MB)

Both replications are done on the host (pure data marshalling), so the device
builds A with plain vector tensor_mul ops (fp16, 2x DVE mode, no partition
broadcast needed, no transposes).  The tensor engine accumulates the output
transposed, out_t[o, b] (2 PSUM banks, one per 128-wide o-half), with the
host-pre-permuted W slice [k, o] as the stationary operand and A[k, b] as the
N=512 moving operand: 256 accumulating matmuls per core.  The fp32 bias is
fused into the PSUM->SBUF eviction as a per-partition tensor_scalar_add.
Everything on-device is fp16 inputs with fp32 PSUM accumulation; measured
max-rel-error vs the fp32 reference is ~4e-4.
"""

import numpy as np

import concourse.bacc as bacc
import concourse.mybir as mybir
import concourse.tile as tile
from concourse.bass_utils import run_bass_kernel_spmd

B, F, P, O = 4096, 128, 128, 256
NCORES = 8
BC = B // NCORES  # 512 batch elements per core
FI, PI = 16, 8  # within-K-tile block: k = fi*PI + pi
FB, PB = F // FI, P // PI  # 8 f-blocks, 16 p-blocks; kt = fb*PB + pb
BT = BC // 128  # 4 batch tiles of 128 per core

_F16 = mybir.dt.float16
_F32 = mybir.dt.float32

_NC_CACHE = {}


VARIANT = "w_stat_512"


def _build_nc(repeat=1, loop_n=0, variant=None):
    """Build + compile the (SPMD, per-core) bass program once.

    repeat>1 re-emits the whole kernel body back-to-back (sharing tile pools,
    so SBUF stays bounded); loop_n>0 additionally wraps the body in a
    hardware For_i loop.  Both are used only by the benchmark harness to
    measure steady-state per-iteration device time via slopes (the axon
    dispatch overhead per call is ~80ms, so single-shot wall time is
    useless).
    """
    if variant is None:
        variant = VARIANT
    key = (repeat, loop_n, variant)
    if key in _NC_CACHE:
        return _NC_CACHE[key]
    n_warm = 0
    if "+warm" in variant:
        variant, warm_str = variant.split("+warm")
        n_warm = int(warm_str)

    nc = bacc.Bacc(
        "TRN2", target_bir_lowering=False, debug=False, num_devices=NCORES
    )

    rep8 = nc.declare_dram_parameter("rep8_x0t", [128, FB, BC], _F16, isOutput=False)
    rep16 = nc.declare_dram_parameter("rep16_xt", [128, PB, BC], _F16, isOutput=False)
    w_re = nc.declare_dram_parameter("w_re", [FB, 128, PB, O], _F16, isOutput=False)
    bias = nc.declare_dram_parameter("bias_col", [O, 1], _F32, isOutput=False)
    # output is stored transposed: out_t[o, b] (host transposes back)
    out = nc.declare_dram_parameter("out_t", [O, BC], _F32, isOutput=True)

    with tile.TileContext(nc) as tc:
        import contextlib

        loop_ctx = (
            tc.For_i(
                0,
                loop_n,
                1,
                hint_engines=(
                    mybir.EngineType.PE,
                    mybir.EngineType.DVE,
                    mybir.EngineType.SP,
                    mybir.EngineType.Activation,
                ),
            )
            if loop_n
            else contextlib.nullcontext()
        )
        with (
            loop_ctx,
            tc.tile_pool(name="inp", bufs=2) as inp,
            tc.tile_pool(name="wp", bufs=2) as wp,
            tc.tile_pool(name="ap", bufs=2) as ap_pool,
            tc.tile_pool(name="op", bufs=2) as op,
            tc.tile_pool(name="ps", bufs=1, space="PSUM") as psp,
        ):
            for _rep in range(repeat):
                # prologue is latency-critical: the first matmuls need only
                # rep8[:, 0], rep16[:, 0:4], and W[0][:, 0:4] — load those
                # first in small chunks (the sync HWDGE queue is FIFO, so
                # emission order = arrival order), stream the rest behind
                rep8_sb = inp.tile([128, FB, BC], _F16, tag="rep8")
                rep16_sb = inp.tile([128, PB, BC], _F16, tag="rep16")
                w_sb_first = wp.tile([128, PB, O], _F16, tag="w", name="w_sb")
                NQ = PB // 4
                nc.sync.dma_start(rep8_sb[:, 0:1, :], rep8[:, 0:1, :])
                for q in range(4):
                    sl = slice(q * NQ, (q + 1) * NQ)
                    nc.sync.dma_start(rep16_sb[:, sl, :], rep16[:, sl, :])
                    nc.sync.dma_start(w_sb_first[:, sl, :], w_re[0][:, sl, :])
                nc.sync.dma_start(rep8_sb[:, 1:FB, :], rep8[:, 1:FB, :])
                bias_sb = inp.tile([128, O // 128], _F32, tag="bias")
                for h in range(O // 128):
                    nc.sync.dma_start(
                        bias_sb[:, h : h + 1], bias[h * 128 : (h + 1) * 128, :]
                    )

                # PE warmup experiment (measured net-negative, default off):
                # dummy matmuls during the prologue to pre-release the HAM
                # clock gate — the cold-rate dummies delay the real stream
                # more than the warm clock saves.
                if n_warm:
                    warm_sb = inp.tile([1, BC], _F16, tag="warm")
                    nc.vector.memset(warm_sb[:], 0.0)
                    warm_ps = psp.tile(
                        [64, BC], _F32, tag="warmps", name="warm_ps"
                    )
                    for _wi in range(n_warm):
                        nc.tensor.matmul(
                            warm_ps[:],
                            warm_sb[:, 0:64],
                            warm_sb[:],
                            start=True,
                            stop=True,
                        )

                if variant == "a_stat":
                    psum_bt_tiles = [
                        psp.tile([128, O], _F32, tag=f"bacc{bt}", name=f"bacc{bt}")
                        for bt in range(BT)
                    ]
                else:
                    # out.T[o, b] accumulators: one full PSUM bank per o-half
                    psum_tiles = [
                        psp.tile([128, BC], _F32, tag=f"acc{h}", name=f"acc{h}")
                        for h in range(O // 128)
                    ]

                for fb in range(FB):
                    if fb == 0:
                        w_sb = w_sb_first
                    else:
                        w_sb = wp.tile([128, PB, O], _F16, tag="w", name="w_sb")
                        nc.sync.dma_start(w_sb[:], w_re[fb])

                    a_sb = ap_pool.tile([128, PB, BC], _F16, tag="a", name="a_sb")
                    nchunk = 4 if fb == 0 else 2
                    csz = PB // nchunk
                    rep8_bc = rep8_sb[:, fb : fb + 1, :].broadcast_to(
                        (128, csz, BC)
                    )
                    for ci in range(nchunk):
                        sl = slice(ci * csz, (ci + 1) * csz)
                        nc.vector.tensor_mul(
                            a_sb[:, sl, :], rep8_bc, rep16_sb[:, sl, :]
                        )

                    first = fb == 0
                    last = fb == FB - 1
                    for pb in range(PB):
                        st = first and pb == 0
                        sp = last and pb == PB - 1
                        if variant == "w_stat_512":
                            # W stationary, full-batch moving (N=512)
                            for h in range(O // 128):
                                nc.tensor.matmul(
                                    psum_tiles[h][:],
                                    w_sb[:, pb, h * 128 : (h + 1) * 128],
                                    a_sb[:, pb, :],
                                    start=st,
                                    stop=sp,
                                )
                        elif variant == "w_stat_256":
                            # W stationary, two N=256 moving halves share
                            # one weight load
                            for h in range(O // 128):
                                for bh in range(2):
                                    nc.tensor.matmul(
                                        psum_tiles[h][
                                            :, bh * 256 : (bh + 1) * 256
                                        ],
                                        w_sb[:, pb, h * 128 : (h + 1) * 128],
                                        a_sb[:, pb, bh * 256 : (bh + 1) * 256],
                                        start=st,
                                        stop=sp,
                                        skip_group_check=True,
                                    )
                        elif variant == "a_stat":
                            # control: A-slices stationary, W moving,
                            # accumulating out[b, o] in 4 b-tile psums.
                            # BENCH-ONLY: no bias, output written scrambled.
                            for bt in range(BT):
                                nc.tensor.matmul(
                                    psum_bt_tiles[bt][:],
                                    a_sb[:, pb, bt * 128 : (bt + 1) * 128],
                                    w_sb[:, pb, :],
                                    start=st,
                                    stop=sp,
                                )
                        else:
                            raise ValueError(variant)

                if variant == "a_stat":
                    flat = out[:].rearrange("o b -> (o b)")
                    for bt in range(BT):
                        out_sb = op.tile([128, O], _F32, tag="out", name="out_sb")
                        nc.scalar.copy(out_sb[:], psum_bt_tiles[bt][:])
                        nc.sync.dma_start(
                            flat[bt * 128 * O : (bt + 1) * 128 * O].rearrange(
                                "(p f) -> p f", p=128
                            ),
                            out_sb[:],
                        )
                else:
                    for h in range(O // 128):
                        out_sb = op.tile([128, BC], _F32, tag="out", name="out_sb")
                        # eviction fused with the (exact, fp32) bias add:
                        # out_t[o, b] = psum[o, b] + bias[o]
                        nc.vector.tensor_scalar_add(
                            out_sb[:], psum_tiles[h][:], bias_sb[:, h : h + 1]
                        )
                        nc.sync.dma_start(
                            out[h * 128 : (h + 1) * 128, :], out_sb[:]
                        )

    nc.compile()
    _NC_CACHE[key] = nc
    return nc


def _prepare_inputs(x0, x, W, b):
    """Host-side marshalling: cast to fp16, transpose, small-factor replicate,
    permute W, and shard the batch across the 8 cores."""
    x0 = np.asarray(x0, dtype=np.float32)
    x = np.asarray(x, dtype=np.float32)
    W = np.asarray(W, dtype=np.float32)
    b = np.asarray(b, dtype=np.float32)

    x0t = np.ascontiguousarray(x0.T.astype(np.float16))  # [F, B]
    xt = np.ascontiguousarray(x.T.astype(np.float16))  # [P, B]

    # rep8[k, fb, bb] = x0t[fb*FI + k//PI, bb]
    rep8 = np.broadcast_to(
        x0t.reshape(FB, FI, B).transpose(1, 0, 2)[:, None, :, :], (FI, PI, FB, B)
    ).reshape(128, FB, B)
    # rep16[k, pb, bb] = xt[pb*PI + k%PI, bb]
    rep16 = np.broadcast_to(
        xt.reshape(PB, PI, B).transpose(1, 0, 2)[None, :, :, :], (FI, PI, PB, B)
    ).reshape(128, PB, B)

    # w_re[fb, k, pb, o] = W[o, fb*FI + k//PI, pb*PI + k%PI]
    w_re = np.ascontiguousarray(
        W.reshape(O, FB, FI, PB, PI).transpose(1, 2, 4, 3, 0).reshape(FB, 128, PB, O)
    ).astype(np.float16)
    bias_col = b.astype(np.float32).reshape(O, 1)

    in_maps = []
    for c in range(NCORES):
        bs = slice(c * BC, (c + 1) * BC)
        in_maps.append(
            {
                "rep8_x0t": np.ascontiguousarray(rep8[:, :, bs]),
                "rep16_xt": np.ascontiguousarray(rep16[:, :, bs]),
                "w_re": w_re,
                "bias_col": bias_col,
            }
        )
    return in_maps


def kernel(x0, x, W, b, _run_kwargs=None):
    nc = _build_nc()
    in_maps = _prepare_inputs(x0, x, W, b)
    res = run_bass_kernel_spmd(
        nc, in_maps, core_ids=list(range(NCORES)), **(_run_kwargs or {})
    )
    # per-core results are out.T shards [O, BC]; assemble + transpose back
    out_t = np.concatenate(
        [res.results[c]["out_t"] for c in range(NCORES)], axis=1
    )  # [O, B]
    if _run_kwargs:
        kernel._last_results = res
    return np.ascontiguousarray(out_t.T).reshape(B, O, 1).astype(np.float32)


# revision 31
# speedup vs baseline: 1.4118x; 1.0362x over previous
"""Trainium2 Bass kernel for the CIN-style bilinear layer:

    out[b, o] = sum_{f,p} x0[b,f] * x[b,p] * W[o,f,p] + bias[o]

i.e. a batch of bilinear forms (outer-product + full-window Conv1d), shapes
B=4096, FIELD=128, H_PREV=128, H_NEXT=256, returned as [B, 256, 1] fp32.

Strategy (data-parallel over 8 NeuronCores, batch sharded 512/core):

The contraction index k=(f,p) has size 16384.  Tile it as 128 K-tiles of 128,
where each K-tile kt=(fb,pb) covers a 16x8 block of (f,p): f = fb*16+fi,
p = pb*8+pi, and the within-tile index is k = fi*8+pi.  The outer-product
operand for one K-tile is then

    A[k, kt, b] = x0T[f(k), b] * xT[p(k), b]

which is an elementwise product of two *small-factor replicated* tensors:
rep8_x0t[k, fb, b] = x0T[fb*16 + k//8, b]   (8x partition replication, 1MB)
rep16_xt[k, pb, b] = xT[pb*8 + k%8, b]      (16x partition replication, 2MB)

Both replications are done on the host (pure data marshalling), so the device
builds A with plain vector tensor_mul ops (fp16, 2x DVE mode, no partition
broadcast needed, no transposes).  The tensor engine accumulates the output
transposed, out_t[o, b] (2 PSUM banks, one per 128-wide o-half), with the
host-pre-permuted W slice [k, o] as the stationary operand and A[k, b] as the
N=512 moving operand: 256 accumulating matmuls per core.  The fp32 bias is
fused into the PSUM->SBUF eviction as a per-partition tensor_scalar_add.
Everything on-device is fp16 inputs with fp32 PSUM accumulation; measured
max-rel-error vs the fp32 reference is ~4e-4.
"""

import numpy as np

import concourse.bacc as bacc
import concourse.mybir as mybir
import concourse.tile as tile
from concourse.bass_utils import run_bass_kernel_spmd

B, F, P, O = 4096, 128, 128, 256
NCORES = 8
BC = B // NCORES  # 512 batch elements per core
FI, PI = 16, 8  # within-K-tile block: k = fi*PI + pi
FB, PB = F // FI, P // PI  # 8 f-blocks, 16 p-blocks; kt = fb*PB + pb
BT = BC // 128  # 4 batch tiles of 128 per core

_F16 = mybir.dt.float16
_F32 = mybir.dt.float32

_NC_CACHE = {}


VARIANT = "w_stat_512"


def _build_nc(repeat=1, loop_n=0, variant=None):
    """Build + compile the (SPMD, per-core) bass program once.

    repeat>1 re-emits the whole kernel body back-to-back (sharing tile pools,
    so SBUF stays bounded); loop_n>0 additionally wraps the body in a
    hardware For_i loop.  Both are used only by the benchmark harness to
    measure steady-state per-iteration device time via slopes (the axon
    dispatch overhead per call is ~80ms, so single-shot wall time is
    useless).
    """
    if variant is None:
        variant = VARIANT
    key = (repeat, loop_n, variant)
    if key in _NC_CACHE:
        return _NC_CACHE[key]
    n_warm = 0
    opts = variant.split("+")
    variant = opts[0]
    nbufs = 2
    tt_chunks = 2
    for o in opts[1:]:
        if o.startswith("warm"):
            n_warm = int(o[4:])
        elif o.startswith("bufs"):
            nbufs = int(o[4:])
        elif o.startswith("ttc"):
            tt_chunks = int(o[3:])

    nc = bacc.Bacc(
        "TRN2", target_bir_lowering=False, debug=False, num_devices=NCORES
    )

    rep8 = nc.declare_dram_parameter("rep8_x0t", [128, FB, BC], _F16, isOutput=False)
    rep16 = nc.declare_dram_parameter("rep16_xt", [128, PB, BC], _F16, isOutput=False)
    w_re = nc.declare_dram_parameter("w_re", [FB, 128, PB, O], _F16, isOutput=False)
    bias = nc.declare_dram_parameter("bias_col", [O, 1], _F32, isOutput=False)
    # output is stored transposed: out_t[o, b] (host transposes back)
    out = nc.declare_dram_parameter("out_t", [O, BC], _F32, isOutput=True)

    with tile.TileContext(nc) as tc:
        import contextlib

        loop_ctx = (
            tc.For_i(
                0,
                loop_n,
                1,
                hint_engines=(
                    mybir.EngineType.PE,
                    mybir.EngineType.DVE,
                    mybir.EngineType.SP,
                    mybir.EngineType.Activation,
                ),
            )
            if loop_n
            else contextlib.nullcontext()
        )
        with (
            loop_ctx,
            tc.tile_pool(name="inp", bufs=2) as inp,
            tc.tile_pool(name="wp", bufs=nbufs) as wp,
            tc.tile_pool(name="ap", bufs=nbufs) as ap_pool,
            tc.tile_pool(name="op", bufs=2) as op,
            tc.tile_pool(name="ps", bufs=1, space="PSUM") as psp,
        ):
            for _rep in range(repeat):
                # prologue is latency-critical: the first matmuls need only
                # rep8[:, 0], rep16[:, 0:4], and W[0][:, 0:4] — load those
                # first in small chunks (the sync HWDGE queue is FIFO, so
                # emission order = arrival order), stream the rest behind
                rep8_sb = inp.tile([128, FB, BC], _F16, tag="rep8")
                rep16_sb = inp.tile([128, PB, BC], _F16, tag="rep16")
                w_sb_first = wp.tile([128, PB, O], _F16, tag="w", name="w_sb")
                NQ = PB // 4
                nc.sync.dma_start(rep8_sb[:, 0:1, :], rep8[:, 0:1, :])
                for q in range(4):
                    sl = slice(q * NQ, (q + 1) * NQ)
                    nc.sync.dma_start(rep16_sb[:, sl, :], rep16[:, sl, :])
                    nc.sync.dma_start(w_sb_first[:, sl, :], w_re[0][:, sl, :])
                nc.sync.dma_start(rep8_sb[:, 1:FB, :], rep8[:, 1:FB, :])
                bias_sb = inp.tile([128, O // 128], _F32, tag="bias")
                for h in range(O // 128):
                    nc.sync.dma_start(
                        bias_sb[:, h : h + 1], bias[h * 128 : (h + 1) * 128, :]
                    )

                # PE warmup experiment (measured net-negative, default off):
                # dummy matmuls during the prologue to pre-release the HAM
                # clock gate — the cold-rate dummies delay the real stream
                # more than the warm clock saves.
                if n_warm:
                    warm_sb = inp.tile([1, BC], _F16, tag="warm")
                    nc.vector.memset(warm_sb[:], 0.0)
                    warm_ps = psp.tile(
                        [64, BC], _F32, tag="warmps", name="warm_ps"
                    )
                    for _wi in range(n_warm):
                        nc.tensor.matmul(
                            warm_ps[:],
                            warm_sb[:, 0:64],
                            warm_sb[:],
                            start=True,
                            stop=True,
                        )

                if variant == "a_stat":
                    psum_bt_tiles = [
                        psp.tile([128, O], _F32, tag=f"bacc{bt}", name=f"bacc{bt}")
                        for bt in range(BT)
                    ]
                else:
                    # out.T[o, b] accumulators: one full PSUM bank per o-half
                    psum_tiles = [
                        psp.tile([128, BC], _F32, tag=f"acc{h}", name=f"acc{h}")
                        for h in range(O // 128)
                    ]

                for fb in range(FB):
                    if fb == 0:
                        w_sb = w_sb_first
                    else:
                        w_sb = wp.tile([128, PB, O], _F16, tag="w", name="w_sb")
                        nc.sync.dma_start(w_sb[:], w_re[fb])

                    a_sb = ap_pool.tile([128, PB, BC], _F16, tag="a", name="a_sb")
                    nchunk = 4 if fb == 0 else tt_chunks
                    csz = PB // nchunk
                    rep8_bc = rep8_sb[:, fb : fb + 1, :].broadcast_to(
                        (128, csz, BC)
                    )
                    for ci in range(nchunk):
                        sl = slice(ci * csz, (ci + 1) * csz)
                        nc.vector.tensor_mul(
                            a_sb[:, sl, :], rep8_bc, rep16_sb[:, sl, :]
                        )

                    first = fb == 0
                    last = fb == FB - 1
                    for pb in range(PB):
                        st = first and pb == 0
                        sp = last and pb == PB - 1
                        if variant == "w_stat_512":
                            # W stationary, full-batch moving (N=512)
                            for h in range(O // 128):
                                nc.tensor.matmul(
                                    psum_tiles[h][:],
                                    w_sb[:, pb, h * 128 : (h + 1) * 128],
                                    a_sb[:, pb, :],
                                    start=st,
                                    stop=sp,
                                )
                        elif variant == "w_stat_256":
                            # W stationary, two N=256 moving halves share
                            # one weight load
                            for h in range(O // 128):
                                for bh in range(2):
                                    nc.tensor.matmul(
                                        psum_tiles[h][
                                            :, bh * 256 : (bh + 1) * 256
                                        ],
                                        w_sb[:, pb, h * 128 : (h + 1) * 128],
                                        a_sb[:, pb, bh * 256 : (bh + 1) * 256],
                                        start=st,
                                        stop=sp,
                                        skip_group_check=True,
                                    )
                        elif variant == "a_stat":
                            # control: A-slices stationary, W moving,
                            # accumulating out[b, o] in 4 b-tile psums.
                            # BENCH-ONLY: no bias, output written scrambled.
                            for bt in range(BT):
                                nc.tensor.matmul(
                                    psum_bt_tiles[bt][:],
                                    a_sb[:, pb, bt * 128 : (bt + 1) * 128],
                                    w_sb[:, pb, :],
                                    start=st,
                                    stop=sp,
                                )
                        else:
                            raise ValueError(variant)

                if variant == "a_stat":
                    flat = out[:].rearrange("o b -> (o b)")
                    for bt in range(BT):
                        out_sb = op.tile([128, O], _F32, tag="out", name="out_sb")
                        nc.scalar.copy(out_sb[:], psum_bt_tiles[bt][:])
                        nc.sync.dma_start(
                            flat[bt * 128 * O : (bt + 1) * 128 * O].rearrange(
                                "(p f) -> p f", p=128
                            ),
                            out_sb[:],
                        )
                else:
                    for h in range(O // 128):
                        out_sb = op.tile([128, BC], _F32, tag="out", name="out_sb")
                        # eviction fused with the (exact, fp32) bias add:
                        # out_t[o, b] = psum[o, b] + bias[o]
                        nc.vector.tensor_scalar_add(
                            out_sb[:], psum_tiles[h][:], bias_sb[:, h : h + 1]
                        )
                        nc.sync.dma_start(
                            out[h * 128 : (h + 1) * 128, :], out_sb[:]
                        )

    nc.compile()
    _NC_CACHE[key] = nc
    return nc


def _prepare_inputs(x0, x, W, b):
    """Host-side marshalling: cast to fp16, transpose, small-factor replicate,
    permute W, and shard the batch across the 8 cores."""
    x0 = np.asarray(x0, dtype=np.float32)
    x = np.asarray(x, dtype=np.float32)
    W = np.asarray(W, dtype=np.float32)
    b = np.asarray(b, dtype=np.float32)

    x0t = np.ascontiguousarray(x0.T.astype(np.float16))  # [F, B]
    xt = np.ascontiguousarray(x.T.astype(np.float16))  # [P, B]

    # rep8[k, fb, bb] = x0t[fb*FI + k//PI, bb]
    rep8 = np.broadcast_to(
        x0t.reshape(FB, FI, B).transpose(1, 0, 2)[:, None, :, :], (FI, PI, FB, B)
    ).reshape(128, FB, B)
    # rep16[k, pb, bb] = xt[pb*PI + k%PI, bb]
    rep16 = np.broadcast_to(
        xt.reshape(PB, PI, B).transpose(1, 0, 2)[None, :, :, :], (FI, PI, PB, B)
    ).reshape(128, PB, B)

    # w_re[fb, k, pb, o] = W[o, fb*FI + k//PI, pb*PI + k%PI]
    w_re = np.ascontiguousarray(
        W.reshape(O, FB, FI, PB, PI).transpose(1, 2, 4, 3, 0).reshape(FB, 128, PB, O)
    ).astype(np.float16)
    bias_col = b.astype(np.float32).reshape(O, 1)

    in_maps = []
    for c in range(NCORES):
        bs = slice(c * BC, (c + 1) * BC)
        in_maps.append(
            {
                "rep8_x0t": np.ascontiguousarray(rep8[:, :, bs]),
                "rep16_xt": np.ascontiguousarray(rep16[:, :, bs]),
                "w_re": w_re,
                "bias_col": bias_col,
            }
        )
    return in_maps


def kernel(x0, x, W, b, _run_kwargs=None):
    nc = _build_nc()
    in_maps = _prepare_inputs(x0, x, W, b)
    res = run_bass_kernel_spmd(
        nc, in_maps, core_ids=list(range(NCORES)), **(_run_kwargs or {})
    )
    # per-core results are out.T shards [O, BC]; assemble + transpose back
    out_t = np.concatenate(
        [res.results[c]["out_t"] for c in range(NCORES)], axis=1
    )  # [O, B]
    if _run_kwargs:
        kernel._last_results = res
    return np.ascontiguousarray(out_t.T).reshape(B, O, 1).astype(np.float32)
